# revision 15
# baseline (speedup 1.0000x reference)
"""Trainium2 Bass kernel for MeshRasterizer (B=2, V=400, F=600, 192x192).

Sharding: 8 cores = 2 batches x 4 row-bands (48 rows x 192 cols = 9216 pixels
per core). Each core rasterizes its batch's mesh over its pixel band.

Per-core pipeline (all on device):
  Stage A: project vertices (PE matmul + DVE), gather face vertices
           (indirect DMA), build per-face edge-coefficient tables.
  Main:    for each 128-pixel tile (72 tiles): w0/w1 planes via K=3 fp32
           matmuls on PE, w2 plane on gpsimd, z plane on DVE (all [128,600]);
           validity-folded key = s*1e30 - z; argmin-z via DVE max/max_index.
  Epilogue: indirect-DMA gather of winning-face rows, per-pixel bary +
           signed edge distances on [128,72] tiles, DMA out.
"""

import sys

sys.path.insert(0, "/opt/trn_rl_repo")

import numpy as np
import concourse.bass as bass
import concourse.bacc as bacc
import concourse.mybir as mybir
from concourse.mybir import AluOpType as Op
from concourse.tile import TileContext
from concourse.bass_utils import run_bass_kernel_spmd

FP = mybir.dt.float32
I32 = mybir.dt.int32
U32 = mybir.dt.uint32
U8 = mybir.dt.uint8

B, V, F, H, W = 2, 400, 600, 192, 192
NCORES = 8
BANDS = 4                      # row-bands per batch
BH = H // BANDS                # 48 rows per band
NPIX = BH * W                  # 9216 pixels per core
NT = NPIX // 128               # 72 pixel tiles per core
FH = F // 2                    # 300, half the faces (PSUM bank limit)
TABW = 20                      # face-table row width (f32)

_CACHE = {}


def _build_nc(debug=False):
    nc = bacc.Bacc("TRN2")

    verts = nc.dram_tensor("verts", [V, 3], FP, kind="ExternalInput")
    Rm = nc.dram_tensor("Rm", [3, 3], FP, kind="ExternalInput")
    Tv = nc.dram_tensor("Tv", [1, 3], FP, kind="ExternalInput")
    faces = nc.dram_tensor("faces", [F, 3], I32, kind="ExternalInput")
    xts = nc.dram_tensor("xts", [3, NPIX], FP, kind="ExternalInput")
    pxc = nc.dram_tensor("pxc", [128, NT], FP, kind="ExternalInput")
    pyc = nc.dram_tensor("pyc", [128, NT], FP, kind="ExternalInput")

    if debug:
        dbg_vsd = nc.dram_tensor("dbg_vsd", [V, 3], FP, kind="ExternalOutput")
        dbg_ftab = nc.dram_tensor("dbg_ftab", [F, TABW], FP, kind="ExternalOutput")
        dbg_c0 = nc.dram_tensor("dbg_c0", [3, F], FP, kind="ExternalOutput")
        dbg_a2b = nc.dram_tensor("dbg_a2b", [128, F], FP, kind="ExternalOutput")
        dbg_w2 = nc.dram_tensor("dbg_w2", [128, F], FP, kind="ExternalOutput")
        dbg_zp = nc.dram_tensor("dbg_zp", [128, F], FP, kind="ExternalOutput")
        dbg_key = nc.dram_tensor("dbg_key", [128, F], FP, kind="ExternalOutput")
        dbg_u = nc.dram_tensor("dbg_u", [128, F], FP, kind="ExternalOutput")
        dbg_v = nc.dram_tensor("dbg_v", [128, F], FP, kind="ExternalOutput")
    pix_o = nc.dram_tensor("pix_o", [NPIX], I32, kind="ExternalOutput")
    zb_o = nc.dram_tensor("zb_o", [NPIX], FP, kind="ExternalOutput")
    bary_o = nc.dram_tensor("bary_o", [NPIX * 3], FP, kind="ExternalOutput")
    dst_o = nc.dram_tensor("dst_o", [NPIX], FP, kind="ExternalOutput")

    from contextlib import ExitStack

    with TileContext(nc) as tc:
        with (
            tc.tile_pool(name="const", bufs=1) as cpool,
            tc.tile_pool(name="stage_a", bufs=1) as sa,
            tc.tile_pool(name="dram", bufs=1, space="DRAM") as dpool,
            tc.tile_pool(name="work", bufs=3) as wk,
            tc.tile_pool(name="res", bufs=1) as res,
        ):
            # stage-A PSUM pools live in their own scope so the main-loop
            # PSUM pool (8 banks) can use the whole PSUM
            stA = ExitStack()
            psA = stA.enter_context(tc.tile_pool(name="psA", bufs=1, space="PSUM"))
            psB = stA.enter_context(tc.tile_pool(name="psB", bufs=2, space="PSUM"))
            # ---------------- Stage A: vertex projection ----------------
            vw_T = sa.tile([3, V], FP)
            nc.sync.dma_start(vw_T[:], verts[:].rearrange("v c -> c v"))
            R_sb = sa.tile([3, 3], FP)
            nc.sync.dma_start(R_sb[:], Rm[:])
            T_col = sa.tile([3, 1], FP)
            nc.sync.dma_start(T_col[:], Tv[:].rearrange("a c -> c a"))

            view_ps = psA.tile([3, V], FP)
            nc.tensor.matmul(view_ps[:], R_sb[:], vw_T[:], start=True, stop=True)
            view_T = sa.tile([3, V], FP)
            nc.vector.tensor_scalar(view_T[:], view_ps[:], T_col[:], None, Op.add)

            # row-extract via DMA (compute engines cannot start mid-partition)
            xr = sa.tile([1, V], FP)
            nc.sync.dma_start(xr[:], view_T[0:1, :])
            yr = sa.tile([1, V], FP)
            nc.sync.dma_start(yr[:], view_T[1:2, :])
            zrow = sa.tile([1, V], FP)
            nc.sync.dma_start(zrow[:], view_T[2:3, :])

            negz = sa.tile([1, V], FP)
            nc.vector.tensor_scalar(negz[:], zrow[:], -1.0, None, Op.mult)
            zabs = sa.tile([1, V], FP)
            nc.vector.tensor_tensor(zabs[:], zrow[:], negz[:], Op.max)
            zsm = sa.tile([1, V], U8)
            nc.vector.tensor_scalar(zsm[:], zabs[:], 1e-2, None, Op.is_lt)
            c001 = sa.tile([1, V], FP)
            nc.vector.memset(c001[:], 1e-2)
            zsafe = sa.tile([1, V], FP)
            nc.vector.select(zsafe[:], zsm[:], c001[:], zrow[:])

            rz = sa.tile([1, V], FP)
            nc.vector.reciprocal(rz[:], zsafe[:])
            xs = sa.tile([1, V], FP)
            nc.vector.tensor_tensor(xs[:], xr[:], rz[:], Op.mult)
            ys = sa.tile([1, V], FP)
            nc.vector.tensor_tensor(ys[:], yr[:], rz[:], Op.mult)

            vsd = dpool.tile([V, 3], FP)
            nc.gpsimd.dma_start(vsd[:, 0:1].rearrange("v j -> j v"), xs[:])
            nc.gpsimd.dma_start(vsd[:, 1:2].rearrange("v j -> j v"), ys[:])
            nc.gpsimd.dma_start(vsd[:, 2:3].rearrange("v j -> j v"), zrow[:])

            # ---------------- Stage A: face gather + coefficients ----------------
            # face f lives at (partition p, chunk c) with f = c*128 + p
            faces_pad = sa.tile([128, 5, 3], I32)
            nc.vector.memset(faces_pad[:], 0)
            nc.sync.dma_start(
                faces_pad[:, 0:4, :],
                faces[0:512, :].rearrange("(c p) k -> p c k", p=128),
            )
            nc.sync.dma_start(faces_pad[0:88, 4, :], faces[512:600, :])
            fv = []
            for k in range(3):
                fvk = sa.tile([128, 5, 3], FP, tag=f"fvk{k}")
                for c in range(5):
                    offk = sa.tile([128, 1], I32, tag=f"offk{k}_{c}", name="offk")
                    nc.vector.tensor_copy(offk[:], faces_pad[:, c, k : k + 1])
                    nc.gpsimd.indirect_dma_start(
                        fvk[:, c, :],
                        None,
                        vsd[:],
                        bass.IndirectOffsetOnAxis(ap=offk[:], axis=0),
                    )
                fv.append(fvk)
            x = [fv[k][:, :, 0] for k in range(3)]
            y = [fv[k][:, :, 1] for k in range(3)]
            z = [fv[k][:, :, 2] for k in range(3)]

            def tt(out, a, b, op):
                nc.vector.tensor_tensor(out, a, b, op)

            def tmp(tag):
                return sa.tile([128, 5], FP, tag=tag, name=tag)

            # edge coeffs (vs reference _edge_coeffs): edge k uses (v_{k+1}, v_{k+2})
            A, Bc, Cc = [], [], []
            for k, (ia, ib) in enumerate(((1, 2), (2, 0), (0, 1))):
                dy = tmp(f"dy{k}")
                tt(dy[:], y[ib], y[ia], Op.subtract)          # dy = yb - ya
                a = tmp(f"a{k}")
                tt(a[:], y[ia], y[ib], Op.subtract)           # a = -dy
                b = tmp(f"b{k}")
                tt(b[:], x[ib], x[ia], Op.subtract)           # b = dx
                t1 = tmp(f"t1_{k}")
                tt(t1[:], dy[:], x[ia], Op.mult)
                t2 = tmp(f"t2_{k}")
                tt(t2[:], b[:], y[ia], Op.mult)
                c = tmp(f"c{k}")
                tt(c[:], t1[:], t2[:], Op.subtract)           # c = dy*ax - dx*ay
                A.append(a)
                Bc.append(b)
                Cc.append(c)

            ar1 = tmp("ar1")
            tt(ar1[:], A[0][:], x[0], Op.mult)
            ar2 = tmp("ar2")
            tt(ar2[:], Bc[0][:], y[0], Op.mult)
            area = tmp("area")
            tt(area[:], ar1[:], ar2[:], Op.add)
            tt(area[:], area[:], Cc[0][:], Op.add)

            nega = tmp("nega")
            nc.vector.tensor_scalar(nega[:], area[:], -1.0, None, Op.mult)
            aabs = tmp("aabs")
            tt(aabs[:], area[:], nega[:], Op.max)
            am = sa.tile([128, 5], U8)
            nc.vector.tensor_scalar(am[:], aabs[:], 1e-8, None, Op.is_gt)
            ceps = tmp("ceps")
            nc.vector.memset(ceps[:], 1e-8)
            asafe = tmp("asafe")
            nc.vector.select(asafe[:], am[:], area[:], ceps[:])
            rasafe = tmp("rasafe")
            nc.vector.reciprocal(rasafe[:], asafe[:])

            # ft columns: 0-8 = a0',b0',c0',a1',b1',c1',a2',b2',c2'
            #             9-14 = x0,y0,x1,y1,x2,y2 ; 15-17 = az',bz',cz'
            ft = sa.tile([128, 5, TABW], FP)
            cneg = tmp("cneg")
            nc.vector.memset(cneg[:], -1e20)
            for k in range(3):
                tt(ft[:, :, 3 * k + 0], A[k][:], rasafe[:], Op.mult)
                tt(ft[:, :, 3 * k + 1], Bc[k][:], rasafe[:], Op.mult)
                cp = tmp(f"cp{k}")
                tt(cp[:], Cc[k][:], rasafe[:], Op.mult)
                # degenerate faces (|area| <= eps) can never be hit
                nc.vector.select(ft[:, :, 3 * k + 2], am[:], cp[:], cneg[:])
            for k in range(3):
                nc.vector.tensor_copy(ft[:, :, 9 + 2 * k], x[k])
                nc.vector.tensor_copy(ft[:, :, 10 + 2 * k], y[k])
            # folded z-plane coeffs: az' = a0'*z0 + a1'*z1 + a2'*z2 (etc.)
            for j in range(3):  # over (a, b, c)
                u = tmp(f"zf{j}")
                tt(u[:], ft[:, :, j], z[0], Op.mult)
                v2 = tmp(f"zg{j}")
                tt(v2[:], ft[:, :, 3 + j], z[1], Op.mult)
                tt(u[:], u[:], v2[:], Op.add)
                tt(v2[:], ft[:, :, 6 + j], z[2], Op.mult)
                tt(ft[:, :, 15 + j], u[:], v2[:], Op.add)

            ftab = dpool.tile([F, TABW], FP)
            nc.gpsimd.dma_start(
                ftab[0:512, :].rearrange("(c p) j -> p c j", p=128),
                ft[:, 0:4, :],
            )
            nc.gpsimd.dma_start(ftab[512:600, :], ft[0:88, 4, :])

            # PE plane coeff rows [3, F] for w0 and w1
            C0_sb = cpool.tile([3, F], FP)
            nc.sync.dma_start(C0_sb[:], ftab[:, 0:3].rearrange("f j -> j f"))
            C1_sb = cpool.tile([3, F], FP)
            nc.sync.dma_start(C1_sb[:], ftab[:, 3:6].rearrange("f j -> j f"))

            # broadcast tables [128, F] for w2 (gpsimd) and z (DVE) planes
            ones = cpool.tile([1, 128], FP)
            nc.vector.memset(ones[:], 1.0)
            bts = {}
            for name, col in (
                ("A2b", 6), ("B2b", 7), ("C2b", 8),
                ("AZb", 15), ("BZb", 16), ("CZb", 17),
            ):
                row = sa.tile([1, F], FP, tag="brow")
                nc.sync.dma_start(
                    row[:], ftab[:, col : col + 1].rearrange("f j -> j f")
                )
                bt = cpool.tile([128, F], FP, tag=name)
                for h in range(2):
                    bps = psB.tile([128, FH], FP, tag="bps")
                    nc.tensor.matmul(
                        bps[:], ones[:], row[:, h * FH : (h + 1) * FH],
                        start=True, stop=True,
                    )
                    nc.vector.tensor_copy(bt[:, h * FH : (h + 1) * FH], bps[:])
                bts[name] = bt

            xts_sb = cpool.tile([3, NPIX], FP)
            nc.sync.dma_start(xts_sb[:], xts[:])
            pxc_sb = cpool.tile([128, NT], FP)
            nc.sync.dma_start(pxc_sb[:], pxc[:])
            pyc_sb = cpool.tile([128, NT], FP)
            nc.sync.dma_start(pyc_sb[:], pyc[:])

            m8all = res.tile([128, NT, 8], FP)
            i8all = res.tile([128, NT, 8], U32)

            stA.close()
            stM = ExitStack()
            mm = stM.enter_context(tc.tile_pool(name="mm", bufs=2, space="PSUM"))

            # ---------------- Main loop over 72 pixel tiles ----------------
            for t in range(NT):
                xcol = pxc_sb[:, t : t + 1]
                ycol = pyc_sb[:, t : t + 1]
                lhsT = xts_sb[:, t * 128 : (t + 1) * 128]

                w0ps = mm.tile([128, 2, 512], FP, tag="w0ps")
                w1ps = mm.tile([128, 2, 512], FP, tag="w1ps")
                for h in range(2):
                    rs = slice(h * FH, (h + 1) * FH)
                    nc.tensor.matmul(
                        w0ps[:, h, 0:FH], lhsT, C0_sb[:, rs], start=True, stop=True
                    )
                    nc.tensor.matmul(
                        w1ps[:, h, 0:FH], lhsT, C1_sb[:, rs], start=True, stop=True
                    )

                # z plane on DVE via folded coeffs
                zt = wk.tile([128, F], FP, tag="zt")
                nc.vector.scalar_tensor_tensor(
                    zt[:], bts["BZb"][:], ycol, bts["CZb"][:], Op.mult, Op.add
                )
                zp = wk.tile([128, F], FP, tag="zp")
                nc.vector.scalar_tensor_tensor(
                    zp[:], bts["AZb"][:], xcol, zt[:], Op.mult, Op.add
                )

                # w2 plane on gpsimd (plain TT ops only; step-0 broadcast APs)
                xcolb = xcol.broadcast_to([128, F])
                ycolb = ycol.broadcast_to([128, F])
                w2t = wk.tile([128, F], FP, tag="w2t")
                nc.gpsimd.tensor_tensor(w2t[:], bts["B2b"][:], ycolb, Op.mult)
                nc.gpsimd.tensor_tensor(w2t[:], w2t[:], bts["C2b"][:], Op.add)
                w2 = wk.tile([128, F], FP, tag="w2")
                nc.gpsimd.tensor_tensor(w2[:], bts["A2b"][:], xcolb, Op.mult)
                nc.gpsimd.tensor_tensor(w2[:], w2[:], w2t[:], Op.add)

                # at most one PSUM operand per DVE op:
                # u = min(w0, 0, w2) ; v = min(z - 1e-6, w1) ; s = min(u, v)
                u = wk.tile([128, F], FP, tag="u")
                v = wk.tile([128, F], FP, tag="v")
                for h in range(2):
                    rs = slice(h * FH, (h + 1) * FH)
                    nc.vector.scalar_tensor_tensor(
                        u[:, rs], w0ps[:, h, 0:FH], 0.0, w2[:, rs], Op.min, Op.min
                    )
                    nc.vector.scalar_tensor_tensor(
                        v[:, rs], zp[:, rs], -1e-6, w1ps[:, h, 0:FH], Op.add, Op.min
                    )
                s = wk.tile([128, F], FP, tag="s")
                nc.vector.tensor_tensor(s[:], u[:], v[:], Op.min)
                key = wk.tile([128, F], FP, tag="key")
                nc.vector.scalar_tensor_tensor(
                    key[:], s[:], 1e30, zp[:], Op.mult, Op.subtract
                )

                nc.vector.max(m8all[:, t], key[:])
                nc.vector.max_index(i8all[:, t], m8all[:, t], key[:])

                if debug and t == 0:
                    nc.gpsimd.dma_start(dbg_w2[:], w2[:])
                    nc.gpsimd.dma_start(dbg_zp[:], zp[:])
                    nc.gpsimd.dma_start(dbg_key[:], key[:])
                    nc.gpsimd.dma_start(dbg_u[:], u[:])
                    nc.gpsimd.dma_start(dbg_v[:], v[:])

            if debug:
                nc.sync.dma_start(dbg_vsd[:], vsd[:])
                nc.sync.dma_start(dbg_ftab[:], ftab[:])
                nc.gpsimd.dma_start(dbg_c0[:], C0_sb[:])
                nc.gpsimd.dma_start(dbg_a2b[:], bts["A2b"][:])

            stM.close()

            # ---------------- Epilogue ----------------
            ep = res  # persistent pool
            idxc = ep.tile([128, NT], U32)
            nc.vector.tensor_copy(idxc[:], i8all[:, :, 0])
            maxv = ep.tile([128, NT], FP)
            nc.vector.tensor_copy(maxv[:], m8all[:, :, 0])

            g = ep.tile([128, NT, TABW], FP)
            for t in range(NT):
                nc.gpsimd.indirect_dma_start(
                    g[:, t, :],
                    None,
                    ftab[:],
                    bass.IndirectOffsetOnAxis(ap=idxc[:, t : t + 1], axis=0),
                )

            hit = ep.tile([128, NT], U8)
            nc.vector.tensor_scalar(hit[:], maxv[:], -1e6, None, Op.is_gt)
            cm1 = ep.tile([128, NT], FP)
            nc.vector.memset(cm1[:], -1.0)

            def ett(a, b, op, tag):
                o = ep.tile([128, NT], FP, tag=tag, name=tag)
                nc.vector.tensor_tensor(o[:], a, b, op)
                return o

            zbv = ep.tile([128, NT], FP)
            nc.vector.tensor_scalar(zbv[:], maxv[:], -1.0, None, Op.mult)
            zb_sb = ep.tile([128, NT], FP)
            nc.vector.select(zb_sb[:], hit[:], zbv[:], cm1[:])

            idxf = ep.tile([128, NT], FP)
            nc.vector.tensor_copy(idxf[:], idxc[:])
            pff = ep.tile([128, NT], FP)
            nc.vector.select(pff[:], hit[:], idxf[:], cm1[:])
            pf_i = ep.tile([128, NT], I32)
            nc.vector.tensor_copy(pf_i[:], pff[:])

            bary_sb = ep.tile([128, NT, 3], FP)
            for k in range(3):
                u = ett(pxc_sb[:], g[:, :, 3 * k + 0], Op.mult, "bu")
                v2 = ett(pyc_sb[:], g[:, :, 3 * k + 1], Op.mult, "bv")
                wv = ett(u[:], v2[:], Op.add, "bw")
                wv = ett(wv[:], g[:, :, 3 * k + 2], Op.add, "bw2")
                nc.vector.select(bary_sb[:, :, k], hit[:], wv[:], cm1[:])

            # signed squared distance to nearest edge of winning face
            segs = []
            for (ca, cb) in ((9, 11), (11, 13), (13, 9)):
                ax, ay = g[:, :, ca], g[:, :, ca + 1]
                bx, by = g[:, :, cb], g[:, :, cb + 1]
                abx = ett(bx, ax, Op.subtract, "abx")
                aby = ett(by, ay, Op.subtract, "aby")
                apx = ett(pxc_sb[:], ax, Op.subtract, "apx")
                apy = ett(pyc_sb[:], ay, Op.subtract, "apy")
                d1 = ett(apx[:], abx[:], Op.mult, "d1")
                d2 = ett(apy[:], aby[:], Op.mult, "d2")
                dot = ett(d1[:], d2[:], Op.add, "dot")
                b1 = ett(abx[:], abx[:], Op.mult, "b1")
                b2 = ett(aby[:], aby[:], Op.mult, "b2")
                ab2 = ett(b1[:], b2[:], Op.add, "ab2")
                den = ep.tile([128, NT], FP, tag="den")
                nc.vector.tensor_scalar(den[:], ab2[:], 1e-8, None, Op.add)
                rden = ep.tile([128, NT], FP, tag="rden", name="rden")
                nc.vector.reciprocal(rden[:], den[:])
                tpar = ett(dot[:], rden[:], Op.mult, "tpar")
                nc.vector.tensor_scalar(tpar[:], tpar[:], 0.0, 1.0, Op.max, Op.min)
                ex = ett(tpar[:], abx[:], Op.mult, "ex")
                ex = ett(ax, ex[:], Op.add, "ex2")
                dx = ett(pxc_sb[:], ex[:], Op.subtract, "dx")
                ey = ett(tpar[:], aby[:], Op.mult, "ey")
                ey = ett(ay, ey[:], Op.add, "ey2")
                dy = ett(pyc_sb[:], ey[:], Op.subtract, "dy")
                q1e = ett(dx[:], dx[:], Op.mult, "q1e")
                q2e = ett(dy[:], dy[:], Op.mult, "q2e")
                segs.append(ett(q1e[:], q2e[:], Op.add, f"seg{ca}"))
            dmin = ett(segs[0][:], segs[1][:], Op.min, "dmin")
            dmin = ett(dmin[:], segs[2][:], Op.min, "dmin2")
            dneg = ep.tile([128, NT], FP)
            nc.vector.tensor_scalar(dneg[:], dmin[:], -1.0, None, Op.mult)
            dst_sb = ep.tile([128, NT], FP)
            nc.vector.select(dst_sb[:], hit[:], dneg[:], cm1[:])

            nc.gpsimd.dma_start(pix_o[:].rearrange("(a b) -> a b", a=128), pf_i[:])
            nc.gpsimd.dma_start(zb_o[:].rearrange("(a b) -> a b", a=128), zb_sb[:])
            nc.gpsimd.dma_start(
                bary_o[:].rearrange("(a b) -> a b", a=128), bary_sb[:]
            )
            nc.gpsimd.dma_start(dst_o[:].rearrange("(a b) -> a b", a=128), dst_sb[:])

    nc.finalize()
    return nc


def _pixel_consts():
    """Per-band constant tensors (pixel NDC grids in the kernel layouts)."""
    out = []
    xs = ((np.arange(W, dtype=np.float32) + 0.5) / W) * 2.0 - 1.0
    ys = ((np.arange(H, dtype=np.float32) + 0.5) / H) * 2.0 - 1.0
    for r in range(BANDS):
        p = np.arange(NPIX)
        py = ys[r * BH + p // W]
        px = xs[p % W]
        # epilogue layout [128, NT]: pixel p = part*NT + t
        pxc = px.reshape(128, NT).astype(np.float32)
        pyc = py.reshape(128, NT).astype(np.float32)
        # matmul layout [3, NPIX]: column t*128 + part <-> pixel part*NT + t
        xts = np.empty((3, NPIX), dtype=np.float32)
        xts[0] = pxc.T.reshape(-1)
        xts[1] = pyc.T.reshape(-1)
        xts[2] = 1.0
        out.append((xts, pxc, pyc))
    return out


def kernel(verts_world, R, T, faces, image_size):
    assert int(image_size) == H
    verts_world = np.ascontiguousarray(verts_world, dtype=np.float32)
    R = np.ascontiguousarray(R, dtype=np.float32)
    T = np.ascontiguousarray(T, dtype=np.float32)
    faces = np.ascontiguousarray(faces, dtype=np.int32)

    if "nc" not in _CACHE:
        _CACHE["nc"] = _build_nc()
        _CACHE["consts"] = _pixel_consts()
    nc = _CACHE["nc"]
    consts = _CACHE["consts"]

    in_maps = []
    for c in range(NCORES):
        b, r = c // BANDS, c % BANDS
        xts, pxc, pyc = consts[r]
        in_maps.append(
            {
                "verts": verts_world[b],
                "Rm": R[b],
                "Tv": T[b : b + 1],
                "faces": faces,
                "xts": xts,
                "pxc": pxc,
                "pyc": pyc,
            }
        )

    rr = run_bass_kernel_spmd(nc, in_maps, core_ids=list(range(NCORES)))
    results = rr.results

    pix = np.empty((B, H, W, 1), dtype=np.int32)
    zb = np.empty((B, H, W, 1), dtype=np.float32)
    bary = np.empty((B, H, W, 1, 3), dtype=np.float32)
    dst = np.empty((B, H, W, 1), dtype=np.float32)
    for c in range(NCORES):
        b, r = c // BANDS, c % BANDS
        rows = slice(r * BH, (r + 1) * BH)
        rc = results[c]
        pix[b, rows, :, 0] = rc["pix_o"].reshape(BH, W)
        zb[b, rows, :, 0] = rc["zb_o"].reshape(BH, W)
        bary[b, rows, :, 0, :] = rc["bary_o"].reshape(BH, W, 3)
        dst[b, rows, :, 0] = rc["dst_o"].reshape(BH, W)
    return pix, zb, bary, dst
